# revision 11
# baseline (speedup 1.0000x reference)
"""Causal self-attention Trainium2 Bass kernel.

Full-input contract: kernel(**inputs) takes the complete tensors from
setup_inputs() and returns the full [B, T, D] output.

Sharding (8 cores): core c handles batch b = c // 2 and head-group
g = c % 2 (8 of the 16 heads).  Data-parallel over B, tensor-parallel over
heads; each core computes a partial out-projection and the host sums the
two per-batch partials (the "all-reduce") and adds b_out.

Per-core device pipeline (all layouts chosen so every matmul contracts over
the SBUF partition dim):
  P2a  qT/kT  [feat, tok]   = W_slice.T-free @ xT       (float32r)
  P2b  v      [tok, feat]   = xT.T-free @ Wv_slice      (float32r) -> bf16
  P3   per head-pair, flash-style over 1024-wide q blocks:
         scoresT[k, q] = kT-tile.T @ qT    (float32r, head pair in
                                            array row-groups 0-63 / 64-127)
         expT = exp(scores / 8)            (ACT, bf16 out, causal mask on
                                            diagonal blocks via triu mult)
         att += expT.T @ v_bf16            (bf16, PSUM accum over k tiles)
         den += expT.T @ ones              (same weights, N=1 matmul)
         att_norm = att * (1/den)          (DVE, per-partition scalar)
  P4   attT = PE-transpose(att)
  P5   out_partial = attT.T @ W_out_slice  (float32r)
"""

import os
import sys

for _p in ("/opt/trn_rl_repo",):
    if _p not in sys.path and os.path.isdir(_p):
        sys.path.append(_p)

import numpy as np

import concourse.bacc as bacc
import concourse.bass as bass
import concourse.tile as tile
from concourse import mybir

F32 = mybir.dt.float32
F32R = mybir.dt.float32r
BF16 = mybir.dt.bfloat16

# Problem constants (nn_CausalSelfAttention: B=4, T=2048, D=1024, H=16)
B = 4
T = 2048
D = 1024
H_TOTAL = 16
DH = 64
N_CORES = 8
NH = H_TOTAL // 2          # heads per core (head-group split of 2)
F = NH * DH                # per-core q (or k or v) feature width = 512
SCALE = 1.0 / float(np.sqrt(DH))


def r(ap):
    """View a float32 AP as float32r for full-rate PE matmuls."""
    return ap.bitcast(F32R)


def build_nc(t=T, d=D, nh=NH, dh=DH):
    """Build the SPMD Bass program (same program on all 8 cores)."""
    f = nh * dh
    dc = d // 128           # contraction chunks over model dim
    tt_n = t // 128         # token tiles
    tr_n = t // 512         # 512-wide token ranges
    np_n = nh // 2          # head pairs
    qbw = min(t, 1024)      # q-block width
    nqb = t // qbw
    qtb = qbw // 128        # q tiles per block
    cqk = 2 * f // 128      # q+k feature chunks (head-pair granularity)

    nc = bacc.Bacc("TRN2", target_bir_lowering=False, debug=False)

    xT = nc.dram_tensor("xT", [d, t], F32R, kind="ExternalInput").ap()
    w = nc.dram_tensor("w", [d, 3 * f], F32R, kind="ExternalInput").ap()
    bqkv = nc.dram_tensor("bqkv", [3 * f], F32, kind="ExternalInput").ap()
    wo = nc.dram_tensor("wo", [f, d], F32R, kind="ExternalInput").ap()
    tri = nc.dram_tensor("tri", [128, 128], BF16, kind="ExternalInput").ap()
    idn = nc.dram_tensor("idn", [128, 128], F32, kind="ExternalInput").ap()
    out = nc.dram_tensor("out", [t, d], F32, kind="ExternalOutput").ap()

    with tile.TileContext(nc) as tc:
        with (
            tc.tile_pool(name="persist", bufs=1) as pp,
            tc.tile_pool(name="work", bufs=1) as kp,
            tc.tile_pool(name="ps", bufs=1, space="PSUM") as ps,
        ):
            xp = tc.alloc_tile_pool(name="xtp", bufs=1)
            wp = tc.alloc_tile_pool(name="wstream", bufs=1)
            # ---- constants / small tensors ----
            tri_sb = pp.tile([128, 128], BF16, tag="tri")
            nc.sync.dma_start(out=tri_sb, in_=tri)
            idn_sb = pp.tile([128, 128], F32, tag="idn")
            nc.sync.dma_start(out=idn_sb, in_=idn)
            ones_sb = pp.tile([128, 1], BF16, tag="ones")
            nc.vector.memset(ones_sb, 1.0)
            # q/k bias, one column per 128-feature chunk
            bqk_sb = pp.tile([128, cqk], F32, tag="bqk")
            nc.sync.dma_start(
                out=bqk_sb, in_=bqkv[0 : 2 * f].rearrange("(c p) -> p c", p=128)
            )
            # v bias broadcast across partitions
            bv_sb = wp.tile([128, f], F32, tag="bv")
            bv_src = bqkv[2 * f : 3 * f]
            bv_bcast = bass.AP(
                tensor=bv_src.tensor,
                offset=bv_src.offset,
                ap=[[0, 128]] + list(bv_src.ap),
            )
            nc.gpsimd.dma_start(out=bv_sb, in_=bv_bcast)

            # ---- xT resident tiles ----
            xt = {}
            for dd in range(dc):
                for tr in range(tr_n):
                    xt[dd, tr] = xp.tile([128, 512], F32R, tag=f"xt_{dd}_{tr}", name=f"xt_{dd}_{tr}")
                    nc.sync.dma_start(
                        out=xt[dd, tr],
                        in_=xT[dd * 128 : (dd + 1) * 128, tr * 512 : (tr + 1) * 512],
                    )

            # ---- persistent intermediates ----
            qk = {}  # transposed q/k: chunk c covers features [128c, 128c+128)
            qk_pools = {}
            vbf = {}  # v in natural [tok, feat] layout, bf16
            for tt in range(tt_n):
                vbf[tt] = pp.tile([128, f], BF16, tag=f"vbf_{tt}", name=f"vbf_{tt}")
            att = {}  # attention out [tok, head*dh], normalized, f32
            for qt in range(tt_n):
                att[qt] = pp.tile([128, f], F32, tag=f"att_{qt}", name=f"att_{qt}")
            attT = {}  # transposed attention out [head*dh, tok] (created late)

            # ---- P2a: one q/k feature chunk -> qk[c, :] ----
            def emit_p2a_chunk(c):
                for tr in range(tr_n):
                    qk[c, tr] = pp.tile(
                        [128, 512], BF16, tag=f"qk_{c}_{tr}", name=f"qk_{c}_{tr}"
                    )
                wch = []
                for dd in range(dc):
                    wt = xp.tile([128, 128], F32R, tag=f"wch_{dd}", bufs=2)
                    nc.sync.dma_start(
                        out=wt, in_=w[dd * 128 : (dd + 1) * 128, c * 128 : (c + 1) * 128]
                    )
                    wch.append(wt)
                for tr in range(tr_n):
                    pm = ps.tile([128, 512], F32, tag="m")
                    for dd in range(dc):
                        nc.tensor.matmul(
                            pm, wch[dd], xt[dd, tr],
                            start=(dd == 0), stop=(dd == dc - 1),
                        )
                    nc.vector.tensor_scalar_add(qk[c, tr], pm, bqk_sb[:, c : c + 1])

            # ---- P2b: one token tile of v (natural layout) -> vbf[tt] ----
            wv = []

            def emit_wv_load():
                for dd in range(dc):
                    wvt = wp.tile([128, f], F32R, tag=f"wv_{dd}")
                    nc.sync.dma_start(
                        out=wvt, in_=w[dd * 128 : (dd + 1) * 128, 2 * f : 3 * f]
                    )
                    wv.append(wvt)

            def emit_p2b_tile(tt):
                tr, ti = divmod(tt, 4)
                pm = ps.tile([128, f], F32, tag="m")
                for dd in range(dc):
                    nc.tensor.matmul(
                        pm,
                        xt[dd, tr][:, ti * 128 : (ti + 1) * 128],
                        wv[dd],
                        start=(dd == 0), stop=(dd == dc - 1),
                    )
                nc.vector.tensor_add(vbf[tt], pm, bv_sb)

            # ---- P3: attention for one head pair, one q block ----
            def emit_p3_block(j, qb):
                """Head pair j (heads 2j, 2j+1), q block qb of width qbw."""
                q_lo = qb * qbw
                kt_max = (qb + 1) * qtb  # k tiles 0..kt_max-1 (causal)
                sc = {}
                av = {}
                for s in range(2):
                    sc[s] = ps.tile([128, qbw], F32, tag=f"s{s}", name=f"sc{s}")
                    av[s] = ps.tile([128, qtb, dh], F32, tag=f"av{s}", name=f"avp{s}")
                den = ps.tile([128, 2, qtb], F32, tag="den")

                # accumulation bookkeeping: first/last matmul into each bank
                av_first = {0: True, 1: True}
                den_first = True
                av_cnt = {0: 0, 1: 0}
                den_cnt = 0
                tot = sum(qtb - max(0, kt - qb * qtb) for kt in range(kt_max))

                for kt in range(kt_max):
                    expx = {}
                    q0 = max(q_lo, kt * 128)
                    for s in range(2):
                        # scoresT[k, q] for head 2j+s
                        kc, kcc = divmod(kt, 4)
                        ktile = qk[f // 128 + j, kc][
                            64 * s : 64 * s + 64, kcc * 128 : (kcc + 1) * 128
                        ]
                        for tr in range(q_lo // 512, (q_lo + qbw) // 512):
                            a = max(q0, tr * 512)
                            b_ = (tr + 1) * 512
                            if a >= b_:
                                continue
                            nc.tensor.matmul(
                                sc[s][:, a - q_lo : b_ - q_lo],
                                ktile,
                                qk[j, tr][64 * s : 64 * s + 64, a - tr * 512 : 512],
                                start=True, stop=True,
                            )
                        # exp (scaled) -> bf16
                        expx[s] = kp.tile([128, qbw], BF16, tag=f"exp{s}", bufs=2, name=f"expx{s}")
                        nc.scalar.activation(
                            expx[s][:, q0 - q_lo :],
                            sc[s][:, q0 - q_lo :],
                            mybir.ActivationFunctionType.Exp,
                            scale=SCALE,
                        )
                        # causal mask on the diagonal block
                        if kt * 128 >= q_lo:
                            off = kt * 128 - q_lo
                            nc.vector.tensor_mul(
                                expx[s][:, off : off + 128],
                                expx[s][:, off : off + 128],
                                tri_sb,
                            )
                    for s in range(2):
                        h = 2 * j + s
                        for qi in range(qtb):
                            qt = qb * qtb + qi
                            if qt < kt:
                                continue
                            lhsT = expx[s][:, qi * 128 : (qi + 1) * 128]
                            av_cnt[s] += 1
                            nc.tensor.matmul(
                                av[s][:, qi, :],
                                lhsT,
                                vbf[kt][:, h * dh : (h + 1) * dh],
                                start=av_first[s], stop=(av_cnt[s] == tot),
                            )
                            av_first[s] = False
                            den_cnt += 1
                            nc.tensor.matmul(
                                den[:, s, qi : qi + 1],
                                lhsT,
                                ones_sb,
                                start=den_first, stop=(den_cnt == 2 * tot),
                            )
                            den_first = False

                # normalize into att[qt]
                for s in range(2):
                    h = 2 * j + s
                    for qi in range(qtb):
                        qt = qb * qtb + qi
                        rc = kp.tile([128, 1], F32, tag="rc", bufs=4)
                        nc.vector.reciprocal(rc, den[:, s, qi : qi + 1])
                        nc.vector.tensor_scalar_mul(
                            att[qt][:, h * dh : (h + 1) * dh], av[s][:, qi, :], rc
                        )

            # ---------------- emission schedule ----------------
            # qk chunks for pair 0 (q chunk j=0, k chunk f//128 + 0)
            emit_p2a_chunk(0)
            emit_p2a_chunk(f // 128 + 0)
            emit_wv_load()
            for tt in range(min(qtb, tt_n)):
                emit_p2b_tile(tt)

            for j in range(np_n):
                for qb in range(nqb):
                    # interleave: remaining v tiles during pair 0,
                    # next pair's q/k chunks during later pairs
                    if j == 0 and qb + 1 < nqb:
                        for tt in range((qb + 1) * qtb, min((qb + 2) * qtb, tt_n)):
                            emit_p2b_tile(tt)
                    if j + 1 < np_n:
                        if qb == 0:
                            emit_p2a_chunk(j + 1)
                        if qb == min(1, nqb - 1):
                            emit_p2a_chunk(f // 128 + j + 1)
                    emit_p3_block(j, qb)
                if j == 0:
                    wp.release()  # wv / bv done (all P2b emitted)
                if j == max(np_n - 2, 0):
                    xp.release()  # last p2a chunk emitted

            # ---- P4/P5 era pool (reuses released xt/qk space) ----
            lp = tc.alloc_tile_pool(name="late", bufs=1)

            # ---- P4: transpose att -> attT ----
            for c in range(f // 128):
                attT[c] = lp.tile([128, t], F32R, tag=f"attT_{c}", name=f"attT_{c}")
            for qt in range(tt_n):
                for c in range(f // 128):
                    trp = ps.tile([128, 128], F32, tag=f"av{c % 2}")
                    nc.tensor.transpose(trp, att[qt][:, c * 128 : (c + 1) * 128], idn_sb)
                    nc.vector.tensor_copy(attT[c][:, qt * 128 : (qt + 1) * 128], trp)

            # ---- P5: out projection ----
            wo_sb = []
            for c in range(f // 128):
                wot = lp.tile([128, d], F32R, tag=f"wo_{c}")
                nc.sync.dma_start(out=wot, in_=wo[c * 128 : (c + 1) * 128, :])
                wo_sb.append(wot)
            for tt in range(tt_n):
                ob = lp.tile([128, d], F32, tag="ob", bufs=3)
                for fh in range((d + 511) // 512):
                    fw = min(512, d - fh * 512)
                    po = ps.tile([128, fw], F32, tag=f"s{fh % 2}", name="po")
                    for c in range(f // 128):
                        nc.tensor.matmul(
                            po,
                            attT[c][:, tt * 128 : (tt + 1) * 128],
                            wo_sb[c][:, fh * 512 : fh * 512 + fw],
                            start=(c == 0), stop=(c == f // 128 - 1),
                        )
                    nc.vector.tensor_copy(ob[:, fh * 512 : fh * 512 + fw], po)
                nc.sync.dma_start(out=out[tt * 128 : (tt + 1) * 128, :], in_=ob)
            lp.release()

    nc.compile()
    return nc


def make_core_inputs(x, W_qkv, b_qkv, W_out, core):
    """Host-side sharding for one core."""
    b, g = core // 2, core % 2
    f = F
    d = D
    xT = np.ascontiguousarray(np.asarray(x[b], dtype=np.float32).T)
    W_qkv = np.asarray(W_qkv, dtype=np.float32)
    cols = np.concatenate(
        [
            np.arange(g * f, (g + 1) * f),
            np.arange(d + g * f, d + (g + 1) * f),
            np.arange(2 * d + g * f, 2 * d + (g + 1) * f),
        ]
    )
    w = np.ascontiguousarray(W_qkv[:, cols])
    bq = np.ascontiguousarray(np.asarray(b_qkv, dtype=np.float32)[cols])
    wo = np.ascontiguousarray(np.asarray(W_out, dtype=np.float32)[g * f : (g + 1) * f, :])
    import ml_dtypes

    tri = np.triu(np.ones((128, 128), np.float32)).astype(ml_dtypes.bfloat16)
    idn = np.eye(128, dtype=np.float32)
    return {"xT": xT, "w": w, "bqkv": bq, "wo": wo, "tri": tri, "idn": idn}


_NC_CACHE = {}


def _get_nc():
    if "nc" not in _NC_CACHE:
        _NC_CACHE["nc"] = build_nc()
    return _NC_CACHE["nc"]


def kernel(x, W_qkv, b_qkv, W_out, b_out):
    from concourse.bass_utils import run_bass_kernel_spmd

    nc = _get_nc()
    in_maps = [
        make_core_inputs(x, W_qkv, b_qkv, W_out, c) for c in range(N_CORES)
    ]
    res = run_bass_kernel_spmd(nc, in_maps, list(range(N_CORES)))
    b_out = np.asarray(b_out, dtype=np.float32)
    outs = []
    for b in range(B):
        outs.append(res.results[2 * b]["out"] + res.results[2 * b + 1]["out"] + b_out)
    return np.stack(outs).astype(np.float32)
